# revision 1
# baseline (speedup 1.0000x reference)
"""CRF loss (dense Gaussian bilateral filter) on 8 Trainium2 NeuronCores.

Math: with feats f_i (coords/ALPHA ++ I/BETA), K[i,j] = exp(-0.5*||f_i-f_j||^2),
s = K @ 1, n = (s+EPS)^-1/2, H = softmax(U), v_c = n*H_c:
    loss = n^T K n - sum_c v_c^T K v_c
(uses sum_c H_c = 1; per-batch block-diagonal K).

Sharding: cores 0-3 -> batch 0, cores 4-7 -> batch 1. Within a batch, each
core owns a 1536-wide slice of the (6144-padded) output-row space and the
full contraction over all 5888-padded j. K tiles are computed once
(PE matmul for dot products in 3-way-split bf16, ScalarE exp) and kept in SBUF as
bf16 [j-part 128, i-free 1536] tiles; row-sums s come from E-stationary
matmuls against a ones vector; s is AllGathered in each 4-core group to
form the normalizer n; the 5-channel weighted filter (n, n*H_c) then streams
the stored K tiles through the PE once more. The scalar loss is reduced on
host from the per-core [5,1536] filter outputs + s.
"""

import numpy as np
import ml_dtypes

import concourse.bass as bass
import concourse.bacc as bacc
import concourse.tile as tile
import concourse.mybir as mybir
import concourse.bass_utils as bass_utils
from concourse.hw_specs import get_activation_tables

ALPHA = 5.0
BETA = 5.0
EPS = 1e-20

B = 2
C = 4
XD = YD = ZD = 18
N = XD * YD * ZD          # 5832
NJ = 5888                 # j padded to 46*128
NJB = 46                  # j blocks of 128
IW = 1536                 # i-rows per core (12*128)
NIC = IW // 128           # 12 i chunks per core
NEG = -120.0              # pad bias => exp -> 0

F32 = mybir.dt.float32
F16 = mybir.dt.float16
BF16 = mybir.dt.bfloat16

TRACE = False
LAST_RESULT = None

_compiled = {}


def _build():
    nc = bacc.Bacc("TRN2", target_bir_lowering=False, debug=False, num_devices=8)

    fhat = nc.dram_tensor("fhat", [39, NJ], BF16, kind="ExternalInput")
    frhs = nc.dram_tensor("frhs", [39, IW], BF16, kind="ExternalInput")
    biasj = nc.dram_tensor("biasj", [128, NJB], F32, kind="ExternalInput")
    h1 = nc.dram_tensor("h1", [128, C * NJB], F32, kind="ExternalInput")
    out = nc.dram_tensor("out", [6, IW], F32, kind="ExternalOutput")
    yt = nc.dram_tensor("yt", [128, 5 * NIC], F32, kind="ExternalOutput")

    with tile.TileContext(nc) as tc:
        with (
            tc.tile_pool(name="const", bufs=1) as cp,
            tc.tile_pool(name="epool", bufs=1) as ep,
            tc.tile_pool(name="spsum", bufs=1, space="PSUM") as sp,
            tc.tile_pool(name="dram", bufs=1, space="DRAM") as dp,
        ):
            fhat_sb = cp.tile([39, NJ], BF16)
            frhs_sb = cp.tile([39, IW], BF16)
            bias_sb = cp.tile([128, NJB], F32)
            h1_sb = cp.tile([128, C * NJB], F32)
            eps_sb = cp.tile([128, 1], F32)
            ones_sb = cp.tile([128, 1], BF16)
            smine_sb = cp.tile([128, NIC], F32)
            ssb = cp.tile([128, 48], F32)
            y_sb = cp.tile([5, IW], F32)
            yt_sb = cp.tile([128, 5 * NIC], F32)
            lnsb = cp.tile([128, 48], F32)
            nsb = cp.tile([128, 48], F32)
            w_sb = cp.tile([128, 5 * NJB], BF16)
            e_sb = ep.tile([128, NJB * IW], BF16)

            bounce = dp.tile([IW], F32)
            gath = dp.tile([4 * IW], F32)

            nc.sync.dma_start(fhat_sb[:, 0:512], fhat[:, 0:512])
            for ch in range(3):
                nc.sync.dma_start(
                    frhs_sb[:, 512 * ch : 512 * (ch + 1)],
                    frhs[:, 512 * ch : 512 * (ch + 1)],
                )
            nc.sync.dma_start(bias_sb[:], biasj[:])
            for a in range(512, NJ, 1536):
                b = min(a + 1536, NJ)
                nc.sync.dma_start(fhat_sb[:, a:b], fhat[:, a:b])
            nc.sync.dma_start(h1_sb[:], h1[:])
            nc.vector.memset(eps_sb[:], EPS)
            nc.vector.memset(ones_sb[:], 1.0)

            # Preload the table set holding BOTH Exp and Ln so no ACT table
            # switches land on the critical path mid-kernel.
            _tabs = list(get_activation_tables("gen3"))
            _nlx = _tabs.index("natural_log_exp_and_others")
            nc.scalar.add_instruction(
                mybir.InstLoadActFuncSet(
                    name=f"I-{nc.next_id()}", act_func_set_id=_nlx
                )
            )

            s_ps = sp.tile([128, NIC], F32)
            # 12 interleaved per-column accumulation groups share this bank;
            # any start=True would clear the whole bank's has_written bits and
            # drop earlier columns' partials. Memset once, accumulate always.
            nc.vector.memset(s_ps[:, :], 0.0)

            # ---- PE warmup group: back-to-back matmuls unthrottle HAM ----
            with tc.tile_pool(name="wrm", bufs=1, space="PSUM") as wp:
                wps = wp.tile([128, 512], F32)
                for i in range(10):
                    nc.tensor.matmul(
                        wps[:, :],
                        fhat_sb[:, 0:128],
                        fhat_sb[:, 0:512],
                        start=(i == 0),
                        stop=(i == 9),
                    )

            # ---- pass A: dot -> exp -> (E stationary) row-sum accumulate ----
            with tc.tile_pool(name="dotp", bufs=2, space="PSUM") as dotp:
                for jb in range(NJB):
                    dps = dotp.tile([128, IW], F32, tag="dot")
                    lw = fhat_sb[:, 128 * jb : 128 * (jb + 1)]
                    for ch in range(3):
                        nc.tensor.matmul(
                            dps[:, 512 * ch : 512 * (ch + 1)],
                            lw,
                            frhs_sb[:, 512 * ch : 512 * (ch + 1)],
                            start=True,
                            stop=True,
                        )
                    nc.scalar.activation(
                        e_sb[:, IW * jb : IW * (jb + 1)],
                        dps[:, :],
                        mybir.ActivationFunctionType.Exp,
                        bias=bias_sb[:, jb : jb + 1],
                        scale=1.0,
                    )
                    for m in range(NIC):
                        nc.tensor.matmul(
                            s_ps[:, m : m + 1],
                            e_sb[:, IW * jb + 128 * m : IW * jb + 128 * (m + 1)],
                            ones_sb[:, :],
                            start=False,
                            stop=(jb == NJB - 1),
                            skip_group_check=True,
                        )

            # ---- AllGather own-row s across the 4-core batch group ----
            nc.vector.tensor_copy(smine_sb[:, :], s_ps[:, :])
            nc.sync.dma_start(
                bounce[:].rearrange("(p c) -> p c", c=NIC), smine_sb[:, :]
            )
            nc.gpsimd.collective_compute(
                "AllGather",
                mybir.AluOpType.bypass,
                replica_groups=[[0, 1, 2, 3], [4, 5, 6, 7]],
                ins=[bounce[:]],
                outs=[gath[:]],
            )
            nc.sync.dma_start(
                ssb[:, :].rearrange("p (r c) -> p r c", c=NIC),
                gath[:].rearrange("(r p c) -> p r c", p=128, c=NIC),
            )
            # Re-warm the PE during the n/W build so pass B starts at 2.4
            # GHz. Gated on the gathered-s load; fp32 matmuls burn ~4 cycles
            # per row so few instructions cover the window.
            with tc.tile_pool(name="wrm2", bufs=1, space="PSUM") as wp2:
                wps2 = wp2.tile([48, 48], F32)
                for i in range(20):
                    nc.tensor.matmul(
                        wps2[:, :],
                        ssb[:, 0:48],
                        ssb[:, 0:48],
                        start=(i == 0),
                        stop=(i == 19),
                    )

            nc.scalar.activation(
                lnsb[:, :],
                ssb[:, :],
                mybir.ActivationFunctionType.Ln,
                bias=eps_sb[:, 0:1],
                scale=1.0,
            )
            nc.scalar.activation(
                nsb[:, :],
                lnsb[:, :],
                mybir.ActivationFunctionType.Exp,
                scale=-0.5,
            )

            # ---- W planes: [n, n*H_0..n*H_3] in bf16, plane-major ----
            nc.vector.tensor_copy(w_sb[:, 0:NJB], nsb[:, 0:NJB])
            for c in range(C):
                nc.vector.tensor_mul(
                    w_sb[:, NJB * (c + 1) : NJB * (c + 2)],
                    nsb[:, 0:NJB],
                    h1_sb[:, NJB * c : NJB * (c + 1)],
                )

            # ---- pass B: Y[5, IW] = W^T E accumulated over all j blocks.
            # Even jb stream E through the PE rhs port (W stationary); odd jb
            # load E chunks through the LDWEIGHTS port and stream the tiny W
            # (output transposed, [i-chunk, 5] per chunk). The two forms use
            # the PE's two independent SBUF read ports, nearly halving the
            # streamed column count on the critical rhs path. Host adds the
            # transposed half back in.
            w_view = w_sb[:, :].rearrange("p (r j) -> p r j", j=NJB)
            evens = [jb for jb in range(NJB) if jb % 3 == 0]
            odds = [jb for jb in range(NJB) if jb % 3 != 0]
            with tc.tile_pool(name="ypool", bufs=1, space="PSUM") as yp:
                y_ps = yp.tile([5, IW], F32)
                yt_ps = yp.tile([128, 5 * NIC], F32)
                nc.vector.memset(yt_ps[:, :], 0.0)
                ne_seen = no_seen = 0
                for jb in range(NJB):
                    if jb in evens:
                        ne_seen += 1
                        lw = w_view[:, :, jb]
                        for ch in range(3):
                            nc.tensor.matmul(
                                y_ps[:, 512 * ch : 512 * (ch + 1)],
                                lw,
                                e_sb[:, IW * jb + 512 * ch : IW * jb + 512 * (ch + 1)],
                                start=(ne_seen == 1),
                                stop=(ne_seen == len(evens)),
                            )
                    else:
                        no_seen += 1
                        for m in range(NIC):
                            nc.tensor.matmul(
                                yt_ps[:, 5 * m : 5 * (m + 1)],
                                e_sb[:, IW * jb + 128 * m : IW * jb + 128 * (m + 1)],
                                w_view[:, :, jb],
                                start=False,
                                stop=(no_seen == len(odds)),
                                skip_group_check=True,
                            )
                nc.vector.tensor_copy(y_sb[:, :], y_ps[:, :])
                nc.vector.tensor_copy(yt_sb[:, :], yt_ps[:, :])
                nc.sync.dma_start(out[0:5, :], y_sb[:, :])
                nc.sync.dma_start(yt[:, :], yt_sb[:, :])
                nc.sync.dma_start(
                    out[5, :].rearrange("(p c) -> p c", c=NIC), smine_sb[:, :]
                )

    nc.compile()
    return nc


def _split3(a):
    """3-way bf16 split: a ~ h + m + l to ~24 mantissa bits."""
    bf = ml_dtypes.bfloat16
    h = a.astype(bf)
    r1 = a - h.astype(np.float32)
    m = r1.astype(bf)
    l = (r1 - m.astype(np.float32)).astype(bf)
    return h, m, l


def kernel(I, U):
    global LAST_RESULT
    if "nc" not in _compiled:
        _compiled["nc"] = _build()
    nc = _compiled["nc"]

    I = np.asarray(I, np.float32)
    U = np.asarray(U, np.float32)

    g = np.arange(XD, dtype=np.float32)
    gx, gy, gz = np.meshgrid(g, g, g, indexing="ij")
    coords = np.stack([gx, gy, gz], 0).reshape(3, N)

    in_maps = []
    host = []  # (H1[4,N], sq not needed beyond device)
    for k in range(8):
        b, r = divmod(k, 4)
        feats = np.concatenate(
            [coords / ALPHA, I[b].reshape(3, N) / BETA], 0
        ).astype(np.float32)  # [6, N]
        sq = (feats.astype(np.float64) ** 2).sum(0)  # [N] f64
        shalf = (-0.5 * sq).astype(np.float32)
        bf = ml_dtypes.bfloat16
        fh, fm, fl = _split3(feats)
        s1, s2, s3 = _split3(shalf)

        one = np.ones((1, N), bf)
        fhat = np.zeros((39, NJ), bf)
        fhat[:, :N] = np.concatenate([fh, fh, fm, fh, fl, fm, one, one, one], 0)

        gi = IW * r + np.arange(IW)
        valid = gi < N
        giv = gi[valid]
        frhs = np.zeros((39, IW), bf)
        frhs[:, valid] = np.concatenate(
            [
                fh[:, giv], fm[:, giv], fh[:, giv], fl[:, giv], fh[:, giv],
                fm[:, giv], s1[None, giv], s2[None, giv], s3[None, giv],
            ],
            0,
        )
        frhs[36, ~valid] = bf(NEG)

        bpad = np.full(NJ, NEG, np.float32)
        bpad[:N] = shalf
        biasj = bpad.reshape(NJB, 128).T.copy()  # [128, NJB]

        uf = U[b].reshape(C, N).astype(np.float64)
        uf = uf - uf.max(0, keepdims=True)
        e = np.exp(uf)
        H1 = (e / e.sum(0, keepdims=True)).astype(np.float32)  # [C, N]
        hpad = np.zeros((C, NJ), np.float32)
        hpad[:, :N] = H1
        h1in = np.concatenate(
            [hpad[c].reshape(NJB, 128).T for c in range(C)], axis=1
        ).copy()  # [128, C*NJB]

        in_maps.append(
            {"fhat": fhat, "frhs": frhs, "biasj": biasj, "h1": h1in}
        )
        host.append((H1, valid, giv, gi))

    res = bass_utils.run_bass_kernel_spmd(
        nc, in_maps, core_ids=list(range(8)), trace=TRACE
    )
    LAST_RESULT = res

    loss = 0.0
    for k in range(8):
        b, r = divmod(k, 4)
        H1, valid, giv, gi = host[k]
        o = res.results[k]["out"].astype(np.float64)
        ytk = res.results[k]["yt"].astype(np.float64)  # [128, 5*NIC]
        # yt[p, 5m+r] = Y_odd[r, 128m+p]
        yodd = ytk.reshape(128, NIC, 5).transpose(2, 1, 0).reshape(5, IW)
        yfull = o[0:5] + yodd
        yv = yfull[:, valid]  # [5, nvalid]
        s = o[5].reshape(128, NIC).T.reshape(IW)[valid]
        n = 1.0 / np.sqrt(s + EPS)
        hv = H1[:, giv].astype(np.float64)  # [C, nvalid]
        loss += (n * yv[0]).sum()
        for c in range(C):
            loss -= (n * hv[c] * yv[1 + c]).sum()
    return np.float32(loss)



# revision 4
# speedup vs baseline: 1.0806x; 1.0806x over previous
"""CRF loss (dense Gaussian bilateral filter) on 8 Trainium2 NeuronCores.

Math: feats f_i = (coords/ALPHA ++ I/BETA), K[i,j] = exp(-0.5||f_i-f_j||^2),
s = K @ 1, n = (s+EPS)^-1/2, H = softmax(U), v_c = n*H_c, Y_c = K v_c:
    loss = sum_c sum_i n_i Y_c,i (1 - H_c,i)
(uses sum_c H_c = 1 so the n^T K n term folds into the C channels).

Sharding (j-sharding, collective-free): cores 0-3 -> batch 0, 4-7 -> batch 1.
Each core owns 12 j-blocks of 128 (1536 rows of the padded-6144 contraction
dim) and streams ALL i (5888 padded). Row-sums s_j = sum_i K[j,i] are then
locally complete: they ride for free on the exp pass via the ScalarE
activation accum_out port. n = rsqrt(s) is computed on the otherwise-idle
VectorE (bit-trick seed + 2 Newton steps), so ScalarE does nothing but the
46*128-per-block exps, which are the kernel's critical path. Pass B
(Y_partial[c,i] = sum_{own j} w_j K[j,i], w = n*H_c) interleaves into the
PE's slack under the ACT-bound pass A, with the 4-plane output packed into
PSUM partition col-groups (i-quarters at partitions 32q..32q+3). The host
sums the 4 per-core partial Y's per batch and does the final f64 reduction.
"""

import numpy as np
import ml_dtypes

import concourse.bass as bass
import concourse.bacc as bacc
import concourse.tile as tile
import concourse.mybir as mybir
import concourse.bass_utils as bass_utils
from concourse.hw_specs import get_activation_tables

ALPHA = 5.0
BETA = 5.0
EPS = 1e-20

B = 2
C = 4
XD = YD = ZD = 18
N = XD * YD * ZD          # 5832
NI = 5888                 # i (free) padded to 46*128
JW = 1536                 # j rows per core (12 blocks of 128)
NJB = 12                  # own j blocks
NEG = -120.0              # pad bias => exp -> 0
QW = NI // 4              # 1472, i-quarter width for PSUM col-group packing
MAGIC = 0x5F3759DF        # fast inverse sqrt seed

F32 = mybir.dt.float32
I32 = mybir.dt.int32
BF16 = mybir.dt.bfloat16

# per-jb PSUM chunk schedule: (buf, ioff, width); A=2048 (4 banks), B=512
# (1 bank). Order chosen so every fill lands in the ACT-busy window of the
# preceding chunks (see pipeline notes in the session log).
CHUNKS = [
    ("B", 0, 512),
    ("A", 512, 2048),
    ("B", 2560, 512),
    ("A", 3072, 2048),
    ("A", 5120, 768),
]

TRACE = False
LAST_RESULT = None

_compiled = {}


def _build():
    nc = bacc.Bacc("TRN2", target_bir_lowering=False, debug=False, num_devices=8)

    flhs = nc.dram_tensor("flhs", [39, JW], BF16, kind="ExternalInput")
    frhs = nc.dram_tensor("frhs", [39, NI], BF16, kind="ExternalInput")
    biasj = nc.dram_tensor("biasj", [128, NJB], F32, kind="ExternalInput")
    h1 = nc.dram_tensor("h1", [128, C * NJB], F32, kind="ExternalInput")
    yq = nc.dram_tensor("yq", [128, QW], F32, kind="ExternalOutput")
    sout = nc.dram_tensor("sout", [128, NJB], F32, kind="ExternalOutput")

    with tile.TileContext(nc) as tc:
        with (
            tc.tile_pool(name="const", bufs=1) as cp,
            tc.tile_pool(name="epool", bufs=1) as ep,
            tc.tile_pool(name="ypsum", bufs=1, space="PSUM") as yp,
            tc.tile_pool(name="apsum", bufs=1, space="PSUM") as ap_,
            tc.tile_pool(name="bpsum", bufs=1, space="PSUM") as bp_,
        ):
            flhs_sb = cp.tile([39, JW], BF16)
            frhs_sb = cp.tile([39, NI], BF16)
            bias_sb = cp.tile([128, NJB], F32)
            h1_sb = cp.tile([128, C * NJB], F32)
            warm_sb = cp.tile([128, 512], BF16)
            s_acc = cp.tile([128, 5 * NJB], F32)
            s_red = cp.tile([128, NJB], F32)
            r_sb = cp.tile([128, NJB], F32)
            t1_sb = cp.tile([128, NJB], F32)
            w_sb = cp.tile([128, C * NJB], BF16)
            y_sb = cp.tile([128, QW], F32)
            e_sb = ep.tile([128, NJB * NI], BF16)

            y_ps = yp.tile([128, 1536], F32)   # 3 banks; quarters in 0:1472
            a_ps = ap_.tile([128, 2048], F32)  # 4 banks
            b_ps = bp_.tile([128, 512], F32)   # 1 bank

            # ---- input DMA, chunked so jb0's first fills start early ----
            nc.sync.dma_start(flhs_sb[:], flhs[:])
            nc.sync.dma_start(frhs_sb[:, 0:2560], frhs[:, 0:2560])
            nc.sync.dma_start(bias_sb[:], biasj[:])
            nc.sync.dma_start(frhs_sb[:, 2560:NI], frhs[:, 2560:NI])
            nc.sync.dma_start(h1_sb[:], h1[:])
            nc.vector.memset(warm_sb[:], 0.0)
            nc.vector.memset(y_ps[:, :], 0.0)

            # Preload the exp table set so no ACT table switch mid-kernel.
            _tabs = list(get_activation_tables("gen3"))
            _nlx = _tabs.index("natural_log_exp_and_others")
            nc.scalar.add_instruction(
                mybir.InstLoadActFuncSet(
                    name=f"I-{nc.next_id()}", act_func_set_id=_nlx
                )
            )

            # ---- PE warmup: sustained matmuls flip HAM to 2.4 GHz ----
            for i in range(7):
                nc.tensor.matmul(
                    b_ps[:, :],
                    warm_sb[:, 0:128],
                    warm_sb[:, 0:512],
                    start=(i == 0),
                    stop=(i == 6),
                )

            h1_v = h1_sb[:, :].rearrange("p (c m) -> p m c", m=NJB)

            def fill(jb, buf, bufoff, ioff, width):
                """dot MMs for one 512-aligned span into PSUM buf."""
                lw = flhs_sb[:, 128 * jb : 128 * (jb + 1)]
                dst = a_ps if buf == "A" else b_ps
                done = 0
                while done < width:
                    w = min(512, width - done)
                    nc.tensor.matmul(
                        dst[:, bufoff + done : bufoff + done + w],
                        lw,
                        frhs_sb[:, ioff + done : ioff + done + w],
                        start=True,
                        stop=True,
                    )
                    done += w

            def act(jb, ci):
                buf, ioff, width = CHUNKS[ci]
                src = a_ps if buf == "A" else b_ps
                nc.scalar.activation(
                    e_sb[:, jb * NI + ioff : jb * NI + ioff + width],
                    src[:, 0:width],
                    mybir.ActivationFunctionType.Exp,
                    bias=bias_sb[:, jb : jb + 1],
                    scale=1.0,
                    accum_out=s_acc[:, 5 * jb + ci : 5 * jb + ci + 1],
                )

            def nw_build(jb):
                """s -> n -> w on VectorE: fast-rsqrt + 2 Newton steps."""
                t = s_red[:, jb : jb + 1]
                nc.vector.tensor_reduce(
                    t,
                    s_acc[:, 5 * jb : 5 * jb + 5],
                    op=mybir.AluOpType.add,
                    axis=mybir.AxisListType.X,
                )
                # pad-j rows have s=0; rsqrt Newton would overflow -> NaN.
                # Valid s >= 247 and the host never reads pad rows.
                nc.vector.tensor_scalar_max(t, t, 1.0)
                r_i = r_sb[:, jb : jb + 1].bitcast(I32)
                nc.vector.tensor_scalar(
                    r_i,
                    t.bitcast(I32),
                    1,
                    -1,
                    op0=mybir.AluOpType.logical_shift_right,
                    op1=mybir.AluOpType.bitwise_xor,
                )
                nc.vector.tensor_scalar_add(r_i, r_i, MAGIC + 1)
                r = r_sb[:, jb : jb + 1]
                t1 = t1_sb[:, jb : jb + 1]
                for _ in range(2):
                    nc.vector.tensor_mul(t1, r, r)
                    nc.vector.tensor_mul(t1, t1, t)
                    nc.vector.tensor_scalar(
                        t1,
                        t1,
                        -0.5,
                        1.5,
                        op0=mybir.AluOpType.mult,
                        op1=mybir.AluOpType.add,
                    )
                    nc.vector.tensor_mul(r, r, t1)
                nc.vector.tensor_mul(
                    w_sb[:, C * jb : C * (jb + 1)],
                    h1_v[:, jb, :],
                    r.broadcast_to([128, C]),
                )

            def passb(jb, last):
                """12 quarter MMs: Y[32q:32q+4, :] += w_jb^T E_jb."""
                lw = w_sb[:, C * jb : C * (jb + 1)]
                for q in range(4):
                    for c0, c1 in ((0, 512), (512, 1024), (1024, QW)):
                        nc.tensor.matmul(
                            y_ps[32 * q : 32 * q + C, c0:c1],
                            lw,
                            e_sb[:, jb * NI + q * QW + c0 : jb * NI + q * QW + c1],
                            start=False,
                            stop=last,
                            skip_group_check=True,
                            tile_position=(0, 32 * q),
                        )

            for jb in range(NJB):
                # dot fills; A-fill MM order lets the early MMs run during
                # the previous jb's tail ACT chunks (see CHUNKS schedule).
                fill(jb, "B", 0, 0, 512)                 # c0
                fill(jb, "A", 1024, 512 + 1024, 1024)    # c1 hi half
                fill(jb, "A", 0, 512, 1024)              # c1 lo half
                act(jb, 0)
                act(jb, 1)
                fill(jb, "B", 0, 2560, 512)              # c2
                act(jb, 2)
                fill(jb, "A", 0, 3072, 2048)             # c3
                act(jb, 3)
                if jb > 0:
                    passb(jb - 1, last=(jb - 1 == NJB - 1))
                fill(jb, "A", 0, 5120, 768)              # c4
                act(jb, 4)
                nw_build(jb)
            passb(NJB - 1, last=True)

            nc.vector.tensor_copy(y_sb[:, :], y_ps[:, 0:QW])
            nc.sync.dma_start(yq[:, :], y_sb[:, :])
            nc.sync.dma_start(sout[:, :], s_red[:, :])

    nc.compile()
    return nc


def _split3(a):
    """3-way bf16 split: a ~ h + m + l to ~24 mantissa bits."""
    bf = ml_dtypes.bfloat16
    h = a.astype(bf)
    r1 = a - h.astype(np.float32)
    m = r1.astype(bf)
    l = (r1 - m.astype(np.float32)).astype(bf)
    return h, m, l


def kernel(I, U):
    global LAST_RESULT
    if "nc" not in _compiled:
        _compiled["nc"] = _build()
    nc = _compiled["nc"]

    I = np.asarray(I, np.float32)
    U = np.asarray(U, np.float32)

    g = np.arange(XD, dtype=np.float32)
    gx, gy, gz = np.meshgrid(g, g, g, indexing="ij")
    coords = np.stack([gx, gy, gz], 0).reshape(3, N)
    bf = ml_dtypes.bfloat16

    in_maps = []
    host = []
    for k in range(8):
        b, r = divmod(k, 4)
        feats = np.concatenate(
            [coords / ALPHA, I[b].reshape(3, N) / BETA], 0
        ).astype(np.float32)  # [6, N]
        sq = (feats.astype(np.float64) ** 2).sum(0)
        shalf = (-0.5 * sq).astype(np.float32)
        fh, fm, fl = _split3(feats)
        s1, s2, s3 = _split3(shalf)

        # rhs: all i (padded to NI); pad cols killed via NEG in the s1 row
        frhs = np.zeros((39, NI), bf)
        frhs[:, :N] = np.concatenate(
            [fh, fm, fh, fl, fh, fm, s1[None], s2[None], s3[None]], 0
        )
        frhs[36, N:] = bf(NEG)

        # lhs: own 1536 j rows; pad j killed via bias NEG
        gj = JW * r + np.arange(JW)
        valid = gj < N
        gjv = gj[valid]
        one = np.ones((1, len(gjv)), bf)
        flhs = np.zeros((39, JW), bf)
        flhs[:, valid] = np.concatenate(
            [
                fh[:, gjv], fh[:, gjv], fm[:, gjv], fh[:, gjv], fl[:, gjv],
                fm[:, gjv], one, one, one,
            ],
            0,
        )

        bpad = np.full(JW, NEG, np.float32)
        bpad[valid] = shalf[gjv]
        biasj = bpad.reshape(NJB, 128).T.copy()  # [128, NJB]

        uf = U[b].reshape(C, N).astype(np.float64)
        uf = uf - uf.max(0, keepdims=True)
        e = np.exp(uf)
        H1 = (e / e.sum(0, keepdims=True)).astype(np.float32)  # [C, N]
        hpad = np.zeros((C, JW), np.float32)
        hpad[:, valid] = H1[:, gjv]
        h1in = np.concatenate(
            [hpad[c].reshape(NJB, 128).T for c in range(C)], axis=1
        ).copy()  # [128, C*NJB]

        in_maps.append(
            {"flhs": flhs, "frhs": frhs, "biasj": biasj, "h1": h1in}
        )
        host.append((H1, valid, gjv))

    res = bass_utils.run_bass_kernel_spmd(
        nc, in_maps, core_ids=list(range(8)), trace=TRACE
    )
    LAST_RESULT = res

    loss = 0.0
    for b in range(B):
        Yb = np.zeros((C, NI), np.float64)
        s_full = np.zeros(N, np.float64)
        for r in range(4):
            k = 4 * b + r
            o = res.results[k]["yq"].astype(np.float64)  # [128, QW]
            for q in range(4):
                Yb[:, q * QW : (q + 1) * QW] += o[32 * q : 32 * q + C, :]
            sk = res.results[k]["sout"].astype(np.float64)  # [128, NJB]
            H1, valid, gjv = host[k]
            s_full[gjv] = sk.T.reshape(JW)[valid]
        H1 = host[4 * b][0].astype(np.float64)
        n = 1.0 / np.sqrt(s_full + EPS)
        acc = np.zeros(N, np.float64)
        for c in range(C):
            acc += Yb[c, :N] * (1.0 - H1[c])
        loss += float((n * acc).sum())
    return np.float32(loss)


# revision 9
# speedup vs baseline: 1.2911x; 1.1948x over previous
"""CRF loss (dense Gaussian bilateral filter) on 8 Trainium2 NeuronCores.

Math: feats f_i = (coords/ALPHA ++ I/BETA), K[i,j] = exp(-0.5||f_i-f_j||^2),
s = K @ 1, n = (s+EPS)^-1/2, H = softmax(U), v_c = n*H_c, Y_c = K v_c:
    loss = sum_c sum_i n_i Y_c,i (1 - H_c,i)
(uses sum_c H_c = 1 so the n^T K n term folds into the C channels).

Sharding (j-sharding, collective-free): cores 0-3 -> batch 0, 4-7 -> batch 1.
Each core owns 12 j-blocks of 128 (1536 rows of the padded-6144 contraction
dim) and streams ALL i (5888 padded). Row-sums s_j = sum_i K[j,i] are then
locally complete: they ride for free on the exp pass via the ScalarE
activation accum_out port. n = rsqrt(s) is computed on the otherwise-idle
VectorE (bit-trick seed + 2 Newton steps), so ScalarE does nothing but the
46*128-per-block exps, which are the kernel's critical path. Pass B
(Y_partial[c,i] = sum_{own j} w_j K[j,i], w = n*H_c) interleaves into the
PE's slack under the ACT-bound pass A, with the 4-plane output packed into
PSUM partition col-groups (i-quarters at partitions 32q..32q+3). The host
sums the 4 per-core partial Y's per batch and does the final f64 reduction.
"""

import numpy as np
import ml_dtypes

import concourse.bass as bass
import concourse.bacc as bacc
import concourse.tile as tile
import concourse.mybir as mybir
import concourse.bass_utils as bass_utils
from concourse.hw_specs import get_activation_tables

ALPHA = 5.0
BETA = 5.0
EPS = 1e-20

B = 2
C = 4
XD = YD = ZD = 18
N = XD * YD * ZD          # 5832
NI = 5888                 # i (free) padded to 46*128
JW = 1536                 # j rows per core (12 blocks of 128)
NJB = 12                  # own j blocks
NEG = -120.0              # pad bias => exp -> 0
QW = NI // 4              # 1472, i-quarter width for PSUM col-group packing
MAGIC = 0x5F3759DF        # fast inverse sqrt seed

F32 = mybir.dt.float32
I32 = mybir.dt.int32
BF16 = mybir.dt.bfloat16

# per-jb PSUM chunk schedule: (buf, ioff, width); A=2048 (4 banks), B=512
# (1 bank). Order chosen so every fill lands in the ACT-busy window of the
# preceding chunks (see pipeline notes in the session log).
CHUNKS = [
    ("B", 0, 512),
    ("A", 512, 2048),
    ("B", 2560, 512),
    ("A", 3072, 2048),
    ("A", 5120, 768),
]

TRACE = False
LAST_RESULT = None

_compiled = {}


def _build():
    nc = bacc.Bacc("TRN2", target_bir_lowering=False, debug=False, num_devices=8)

    flhs = nc.dram_tensor("flhs", [39, JW], BF16, kind="ExternalInput")
    frhs = nc.dram_tensor("frhs", [39, NI], BF16, kind="ExternalInput")
    biasj = nc.dram_tensor("biasj", [128, NJB], F32, kind="ExternalInput")
    h1 = nc.dram_tensor("h1", [128, C * NJB], F32, kind="ExternalInput")
    yq = nc.dram_tensor("yq", [128, QW], F32, kind="ExternalOutput")
    sout = nc.dram_tensor("sout", [128, NJB], F32, kind="ExternalOutput")

    with tile.TileContext(nc) as tc:
        with (
            tc.tile_pool(name="const", bufs=1) as cp,
            tc.tile_pool(name="epool", bufs=1) as ep,
            tc.tile_pool(name="ypsum", bufs=1, space="PSUM") as yp,
            tc.tile_pool(name="apsum", bufs=1, space="PSUM") as ap_,
            tc.tile_pool(name="bpsum", bufs=1, space="PSUM") as bp_,
        ):
            flhs_sb = cp.tile([39, JW], BF16)
            frhs_sb = cp.tile([39, NI], BF16)
            bias_sb = cp.tile([128, NJB], F32)
            h1_sb = cp.tile([128, C * NJB], F32)
            warm_sb = cp.tile([128, 512], BF16)
            s_acc = cp.tile([128, 5 * NJB], F32)
            s_red = cp.tile([128, NJB], F32)
            r_sb = cp.tile([128, NJB], F32)
            t1_sb = cp.tile([128, NJB], F32)
            w_sb = cp.tile([128, C * NJB], BF16)
            y_sb = cp.tile([128, QW], F32)
            dum_sb = cp.tile([128, 1], BF16)
            e_sb = ep.tile([128, NJB * NI], BF16)

            y_ps = yp.tile([128, 1536], F32)   # 3 banks; quarters in 0:1472
            a_ps = ap_.tile([128, 2048], F32)  # 4 banks
            b_ps = bp_.tile([128, 512], F32)   # 1 bank

            # ---- input DMA, chunked so jb0's first fills start early ----
            nc.sync.dma_start(flhs_sb[:], flhs[:])
            nc.sync.dma_start(frhs_sb[:, 0:2560], frhs[:, 0:2560])
            nc.sync.dma_start(bias_sb[:], biasj[:])
            nc.sync.dma_start(frhs_sb[:, 2560:NI], frhs[:, 2560:NI])
            nc.sync.dma_start(h1_sb[:], h1[:])
            nc.vector.memset(warm_sb[:], 0.0)
            nc.vector.memset(y_ps[:, :], 0.0)

            # Preload the exp table set so no ACT table switch mid-kernel.
            _tabs = list(get_activation_tables("gen3"))
            _nlx = _tabs.index("natural_log_exp_and_others")
            nc.scalar.add_instruction(
                mybir.InstLoadActFuncSet(
                    name=f"I-{nc.next_id()}", act_func_set_id=_nlx
                )
            )

            # ---- PE warmup: >=4us of sustained matmuls flip HAM to 2.4 GHz
            NWARM = 12
            for i in range(NWARM):
                nc.tensor.matmul(
                    b_ps[:, :],
                    warm_sb[:, 0:128],
                    warm_sb[:, 0:512],
                    start=(i == 0),
                    stop=(i == NWARM - 1),
                )

            h1_v = h1_sb[:, :].rearrange("p (c m) -> p m c", m=NJB)

            def fill(jb, buf, bufoff, ioff, width):
                """dot MMs for one 512-aligned span into PSUM buf."""
                lw = flhs_sb[:, 128 * jb : 128 * (jb + 1)]
                dst = a_ps if buf == "A" else b_ps
                done = 0
                while done < width:
                    w = min(512, width - done)
                    nc.tensor.matmul(
                        dst[:, bufoff + done : bufoff + done + w],
                        lw,
                        frhs_sb[:, ioff + done : ioff + done + w],
                        start=True,
                        stop=True,
                    )
                    done += w

            def act(jb, ci):
                # accum_out drops ACT to ~1.0 GHz and adds a ~341ns
                # READ_ACCUMULATOR -- only worth it on the small chunks;
                # the idle DVE sums the two 2048-col chunks instead.
                buf, ioff, width = CHUNKS[ci]
                src = a_ps if buf == "A" else b_ps
                acc = (
                    s_acc[:, 5 * jb + ci : 5 * jb + ci + 1]
                    if ci in (0, 2, 4)
                    else None
                )
                nc.scalar.activation(
                    e_sb[:, jb * NI + ioff : jb * NI + ioff + width],
                    src[:, 0:width],
                    mybir.ActivationFunctionType.Exp,
                    bias=bias_sb[:, jb : jb + 1],
                    scale=1.0,
                    accum_out=acc,
                )

            def dve_sum(jb, ci):
                _, ioff, width = CHUNKS[ci]
                e = e_sb[:, jb * NI + ioff : jb * NI + ioff + width]
                nc.vector.tensor_reduce(
                    s_acc[:, 5 * jb + ci : 5 * jb + ci + 1],
                    e,
                    op=mybir.AluOpType.add,
                    axis=mybir.AxisListType.X,
                )

            def nw_build(jb):
                """s -> n -> w on VectorE: fast-rsqrt + 2 Newton steps."""
                t = s_red[:, jb : jb + 1]
                nc.vector.tensor_reduce(
                    t,
                    s_acc[:, 5 * jb : 5 * jb + 5],
                    op=mybir.AluOpType.add,
                    axis=mybir.AxisListType.X,
                )
                # pad-j rows have s=0; rsqrt Newton would overflow -> NaN.
                # Valid s >= 247 and the host never reads pad rows.
                nc.vector.tensor_scalar_max(t, t, 1.0)
                r_i = r_sb[:, jb : jb + 1].bitcast(I32)
                nc.vector.tensor_scalar(
                    r_i,
                    t.bitcast(I32),
                    1,
                    -1,
                    op0=mybir.AluOpType.logical_shift_right,
                    op1=mybir.AluOpType.bitwise_xor,
                )
                nc.vector.tensor_scalar_add(r_i, r_i, MAGIC + 1)
                r = r_sb[:, jb : jb + 1]
                t1 = t1_sb[:, jb : jb + 1]
                for _ in range(2):
                    nc.vector.tensor_mul(t1, r, r)
                    nc.vector.tensor_mul(t1, t1, t)
                    nc.vector.tensor_scalar(
                        t1,
                        t1,
                        -0.5,
                        1.5,
                        op0=mybir.AluOpType.mult,
                        op1=mybir.AluOpType.add,
                    )
                    nc.vector.tensor_mul(r, r, t1)
                nc.vector.tensor_mul(
                    w_sb[:, C * jb : C * (jb + 1)],
                    h1_v[:, jb, :],
                    r.broadcast_to([128, C]),
                )

            def passb_mms(jb, last):
                """12 quarter MMs: Y[32q:32q+4, :] += w_jb^T E_jb."""
                lw = w_sb[:, C * jb : C * (jb + 1)]
                for q in range(4):
                    for c0, c1 in ((0, 512), (512, 1024), (1024, QW)):
                        yield lambda q=q, c0=c0, c1=c1: nc.tensor.matmul(
                            y_ps[32 * q : 32 * q + C, c0:c1],
                            lw,
                            e_sb[:, jb * NI + q * QW + c0 : jb * NI + q * QW + c1],
                            start=False,
                            stop=last,
                            skip_group_check=True,
                            tile_position=(0, 32 * q),
                        )

            def emit(gen, k):
                for _ in range(k):
                    f = next(gen, None)
                    if f is None:
                        return
                    f()

            pb = iter(())
            for jb in range(NJB):
                # dot fills; A-fill MM order lets the early MMs run during
                # the previous jb's tail ACT chunks (see CHUNKS schedule).
                # passB MMs of jb-1 are spread into the PE FIFO at points
                # where ACT is busy, keeping the PE dense for HAM.
                fill(jb, "B", 0, 0, 512)                 # c0
                fill(jb, "A", 1024, 512 + 1024, 1024)    # c1 hi half
                fill(jb, "A", 0, 512, 1024)              # c1 lo half
                act(jb, 0)
                act(jb, 1)
                fill(jb, "B", 0, 2560, 512)              # c2
                emit(pb, 4)
                act(jb, 2)
                dve_sum(jb, 1)
                fill(jb, "A", 0, 3072, 2048)             # c3
                emit(pb, 4)
                act(jb, 3)
                fill(jb, "A", 0, 5120, 768)              # c4
                emit(pb, 12)
                act(jb, 4)
                dve_sum(jb, 3)
                nw_build(jb)
                pb = passb_mms(jb, last=(jb == NJB - 1))
            emit(pb, 12)

            nc.vector.tensor_copy(y_sb[:, :], y_ps[:, 0:QW])
            nc.sync.dma_start(yq[:, :], y_sb[:, :])
            nc.sync.dma_start(sout[:, :], s_red[:, :])

    nc.compile()
    return nc


def _split3(a):
    """3-way bf16 split: a ~ h + m + l to ~24 mantissa bits."""
    bf = ml_dtypes.bfloat16
    h = a.astype(bf)
    r1 = a - h.astype(np.float32)
    m = r1.astype(bf)
    l = (r1 - m.astype(np.float32)).astype(bf)
    return h, m, l


def kernel(I, U):
    global LAST_RESULT
    if "nc" not in _compiled:
        _compiled["nc"] = _build()
    nc = _compiled["nc"]

    I = np.asarray(I, np.float32)
    U = np.asarray(U, np.float32)

    g = np.arange(XD, dtype=np.float32)
    gx, gy, gz = np.meshgrid(g, g, g, indexing="ij")
    coords = np.stack([gx, gy, gz], 0).reshape(3, N)
    bf = ml_dtypes.bfloat16

    in_maps = []
    host = []
    for k in range(8):
        b, r = divmod(k, 4)
        feats = np.concatenate(
            [coords / ALPHA, I[b].reshape(3, N) / BETA], 0
        ).astype(np.float32)  # [6, N]
        sq = (feats.astype(np.float64) ** 2).sum(0)
        shalf = (-0.5 * sq).astype(np.float32)
        fh, fm, fl = _split3(feats)
        s1, s2, s3 = _split3(shalf)

        # rhs: all i (padded to NI); pad cols killed via NEG in the s1 row
        frhs = np.zeros((39, NI), bf)
        frhs[:, :N] = np.concatenate(
            [fh, fm, fh, fl, fh, fm, s1[None], s2[None], s3[None]], 0
        )
        frhs[36, N:] = bf(NEG)

        # lhs: own 1536 j rows; pad j killed via bias NEG
        gj = JW * r + np.arange(JW)
        valid = gj < N
        gjv = gj[valid]
        one = np.ones((1, len(gjv)), bf)
        flhs = np.zeros((39, JW), bf)
        flhs[:, valid] = np.concatenate(
            [
                fh[:, gjv], fh[:, gjv], fm[:, gjv], fh[:, gjv], fl[:, gjv],
                fm[:, gjv], one, one, one,
            ],
            0,
        )

        bpad = np.full(JW, NEG, np.float32)
        bpad[valid] = shalf[gjv]
        biasj = bpad.reshape(NJB, 128).T.copy()  # [128, NJB]

        uf = U[b].reshape(C, N).astype(np.float64)
        uf = uf - uf.max(0, keepdims=True)
        e = np.exp(uf)
        H1 = (e / e.sum(0, keepdims=True)).astype(np.float32)  # [C, N]
        hpad = np.zeros((C, JW), np.float32)
        hpad[:, valid] = H1[:, gjv]
        h1in = np.concatenate(
            [hpad[c].reshape(NJB, 128).T for c in range(C)], axis=1
        ).copy()  # [128, C*NJB]

        in_maps.append(
            {"flhs": flhs, "frhs": frhs, "biasj": biasj, "h1": h1in}
        )
        host.append((H1, valid, gjv))

    res = bass_utils.run_bass_kernel_spmd(
        nc, in_maps, core_ids=list(range(8)), trace=TRACE
    )
    LAST_RESULT = res

    loss = 0.0
    for b in range(B):
        Yb = np.zeros((C, NI), np.float64)
        s_full = np.zeros(N, np.float64)
        for r in range(4):
            k = 4 * b + r
            o = res.results[k]["yq"].astype(np.float64)  # [128, QW]
            for q in range(4):
                Yb[:, q * QW : (q + 1) * QW] += o[32 * q : 32 * q + C, :]
            sk = res.results[k]["sout"].astype(np.float64)  # [128, NJB]
            H1, valid, gjv = host[k]
            s_full[gjv] = sk.T.reshape(JW)[valid]
        H1 = host[4 * b][0].astype(np.float64)
        n = 1.0 / np.sqrt(s_full + EPS)
        acc = np.zeros(N, np.float64)
        for c in range(C):
            acc += Yb[c, :N] * (1.0 - H1[c])
        loss += float((n * acc).sum())
    return np.float32(loss)


# revision 10
# speedup vs baseline: 1.3113x; 1.0156x over previous
"""CRF loss (dense Gaussian bilateral filter) on 8 Trainium2 NeuronCores.

Math: feats f_i = (coords/ALPHA ++ I/BETA), K[i,j] = exp(-0.5||f_i-f_j||^2),
s = K @ 1, n = (s+EPS)^-1/2, H = softmax(U), v_c = n*H_c, Y_c = K v_c:
    loss = sum_c sum_i n_i Y_c,i (1 - H_c,i)
(uses sum_c H_c = 1 so the n^T K n term folds into the C channels).

Sharding (j-sharding, collective-free): cores 0-3 -> batch 0, 4-7 -> batch 1.
Each core owns 12 j-blocks of 128 (1536 rows of the padded-6144 contraction
dim) and streams ALL i (5888 padded). Row-sums s_j = sum_i K[j,i] are then
locally complete: they ride for free on the exp pass via the ScalarE
activation accum_out port. n = rsqrt(s) is computed on the otherwise-idle
VectorE (bit-trick seed + 2 Newton steps), so ScalarE does nothing but the
46*128-per-block exps, which are the kernel's critical path. Pass B
(Y_partial[c,i] = sum_{own j} w_j K[j,i], w = n*H_c) interleaves into the
PE's slack under the ACT-bound pass A, with the 4-plane output packed into
PSUM partition col-groups (i-quarters at partitions 32q..32q+3). The host
sums the 4 per-core partial Y's per batch and does the final f64 reduction.
"""

import numpy as np
import ml_dtypes

import concourse.bass as bass
import concourse.bacc as bacc
import concourse.tile as tile
import concourse.mybir as mybir
import concourse.bass_utils as bass_utils
from concourse.hw_specs import get_activation_tables

ALPHA = 5.0
BETA = 5.0
EPS = 1e-20

B = 2
C = 4
XD = YD = ZD = 18
N = XD * YD * ZD          # 5832
NI = 5888                 # i (free) padded to 46*128
JW = 1536                 # j rows per core (12 blocks of 128)
NJB = 12                  # own j blocks
NEG = -120.0              # pad bias => exp -> 0
QW = NI // 4              # 1472, i-quarter width for PSUM col-group packing
MAGIC = 0x5F3759DF        # fast inverse sqrt seed

F32 = mybir.dt.float32
I32 = mybir.dt.int32
BF16 = mybir.dt.bfloat16

# per-jb PSUM chunk schedule: (buf, ioff, width); A=2048 (4 banks), B=512
# (1 bank). Order chosen so every fill lands in the ACT-busy window of the
# preceding chunks (see pipeline notes in the session log).
CHUNKS = [
    ("B", 0, 512),
    ("A", 512, 2048),
    ("B", 2560, 512),
    ("A", 3072, 2048),
    ("A", 5120, 768),
]

TRACE = False
LAST_RESULT = None

_compiled = {}


def _build():
    nc = bacc.Bacc("TRN2", target_bir_lowering=False, debug=False, num_devices=8)

    flhs = nc.dram_tensor("flhs", [39, JW], BF16, kind="ExternalInput")
    frhs = nc.dram_tensor("frhs", [39, NI], BF16, kind="ExternalInput")
    biasj = nc.dram_tensor("biasj", [128, NJB], F32, kind="ExternalInput")
    h1 = nc.dram_tensor("h1", [128, C * NJB], F32, kind="ExternalInput")
    yq = nc.dram_tensor("yq", [128, QW], F32, kind="ExternalOutput")
    sout = nc.dram_tensor("sout", [128, NJB], F32, kind="ExternalOutput")

    with tile.TileContext(nc) as tc:
        with (
            tc.tile_pool(name="const", bufs=1) as cp,
            tc.tile_pool(name="epool", bufs=1) as ep,
            tc.tile_pool(name="ypsum", bufs=1, space="PSUM") as yp,
            tc.tile_pool(name="apsum", bufs=1, space="PSUM") as ap_,
            tc.tile_pool(name="bpsum", bufs=1, space="PSUM") as bp_,
        ):
            flhs_sb = cp.tile([39, JW], BF16)
            frhs_sb = cp.tile([39, NI], BF16)
            bias_sb = cp.tile([128, NJB], F32)
            h1_sb = cp.tile([128, C * NJB], F32)
            warm_sb = cp.tile([128, 512], BF16)
            s_acc = cp.tile([128, 5 * NJB], F32)
            s_red = cp.tile([128, NJB], F32)
            r_sb = cp.tile([128, NJB], F32)
            t1_sb = cp.tile([128, NJB], F32)
            w_sb = cp.tile([128, C * NJB], BF16)
            y_sb = cp.tile([128, QW], F32)
            dum_sb = cp.tile([128, 1], BF16)
            e_sb = ep.tile([128, NJB * NI], BF16)

            y_ps = yp.tile([128, 1536], F32)   # 3 banks; quarters in 0:1472
            a_ps = ap_.tile([128, 2048], F32)  # 4 banks
            b_ps = bp_.tile([128, 512], F32)   # 1 bank

            # ---- input DMA, chunked so jb0's first fills start early ----
            nc.sync.dma_start(flhs_sb[:], flhs[:])
            nc.sync.dma_start(frhs_sb[:, 0:2560], frhs[:, 0:2560])
            nc.sync.dma_start(bias_sb[:], biasj[:])
            nc.sync.dma_start(frhs_sb[:, 2560:NI], frhs[:, 2560:NI])
            nc.sync.dma_start(h1_sb[:], h1[:])
            nc.vector.memset(warm_sb[:], 0.0)
            nc.vector.memset(y_ps[:, :], 0.0)

            # Preload the exp table set so no ACT table switch mid-kernel.
            _tabs = list(get_activation_tables("gen3"))
            _nlx = _tabs.index("natural_log_exp_and_others")
            nc.scalar.add_instruction(
                mybir.InstLoadActFuncSet(
                    name=f"I-{nc.next_id()}", act_func_set_id=_nlx
                )
            )

            # ---- PE warmup: >=6us of sustained matmuls flip HAM to 2.4 GHz
            # (measured on this box: flip after ~14 back-to-back 512-col MMs)
            NWARM = 16
            for i in range(NWARM):
                nc.tensor.matmul(
                    b_ps[:, :],
                    warm_sb[:, 0:128],
                    warm_sb[:, 0:512],
                    start=(i == 0),
                    stop=(i == NWARM - 1),
                )

            h1_v = h1_sb[:, :].rearrange("p (c m) -> p m c", m=NJB)

            def fill(jb, buf, bufoff, ioff, width):
                """dot MMs for one 512-aligned span into PSUM buf."""
                lw = flhs_sb[:, 128 * jb : 128 * (jb + 1)]
                dst = a_ps if buf == "A" else b_ps
                done = 0
                while done < width:
                    w = min(512, width - done)
                    nc.tensor.matmul(
                        dst[:, bufoff + done : bufoff + done + w],
                        lw,
                        frhs_sb[:, ioff + done : ioff + done + w],
                        start=True,
                        stop=True,
                    )
                    done += w

            def act(jb, ci):
                # accum_out drops ACT to ~1.0 GHz and adds a ~341ns
                # READ_ACCUMULATOR -- only worth it on the small chunks;
                # the idle DVE sums the two 2048-col chunks instead.
                buf, ioff, width = CHUNKS[ci]
                src = a_ps if buf == "A" else b_ps
                acc = (
                    s_acc[:, 5 * jb + ci : 5 * jb + ci + 1]
                    if ci in (0, 2, 4)
                    else None
                )
                nc.scalar.activation(
                    e_sb[:, jb * NI + ioff : jb * NI + ioff + width],
                    src[:, 0:width],
                    mybir.ActivationFunctionType.Exp,
                    bias=bias_sb[:, jb : jb + 1],
                    scale=1.0,
                    accum_out=acc,
                )

            def dve_sum(jb, ci):
                _, ioff, width = CHUNKS[ci]
                e = e_sb[:, jb * NI + ioff : jb * NI + ioff + width]
                nc.vector.tensor_reduce(
                    s_acc[:, 5 * jb + ci : 5 * jb + ci + 1],
                    e,
                    op=mybir.AluOpType.add,
                    axis=mybir.AxisListType.X,
                )

            def nw_build(jb):
                """s -> n -> w on VectorE: fast-rsqrt + 2 Newton steps."""
                t = s_red[:, jb : jb + 1]
                nc.vector.tensor_reduce(
                    t,
                    s_acc[:, 5 * jb : 5 * jb + 5],
                    op=mybir.AluOpType.add,
                    axis=mybir.AxisListType.X,
                )
                # pad-j rows have s=0; rsqrt Newton would overflow -> NaN.
                # Valid s >= 247 and the host never reads pad rows.
                nc.vector.tensor_scalar_max(t, t, 1.0)
                r_i = r_sb[:, jb : jb + 1].bitcast(I32)
                nc.vector.tensor_scalar(
                    r_i,
                    t.bitcast(I32),
                    1,
                    -1,
                    op0=mybir.AluOpType.logical_shift_right,
                    op1=mybir.AluOpType.bitwise_xor,
                )
                nc.vector.tensor_scalar_add(r_i, r_i, MAGIC + 1)
                r = r_sb[:, jb : jb + 1]
                t1 = t1_sb[:, jb : jb + 1]
                for _ in range(2):
                    nc.vector.tensor_mul(t1, r, r)
                    nc.vector.tensor_mul(t1, t1, t)
                    nc.vector.tensor_scalar(
                        t1,
                        t1,
                        -0.5,
                        1.5,
                        op0=mybir.AluOpType.mult,
                        op1=mybir.AluOpType.add,
                    )
                    nc.vector.tensor_mul(r, r, t1)
                nc.vector.tensor_mul(
                    w_sb[:, C * jb : C * (jb + 1)],
                    h1_v[:, jb, :],
                    r.broadcast_to([128, C]),
                )

            def passb_mms(jb, last):
                """12 quarter MMs: Y[32q:32q+4, :] += w_jb^T E_jb."""
                lw = w_sb[:, C * jb : C * (jb + 1)]
                for q in range(4):
                    for c0, c1 in ((0, 512), (512, 1024), (1024, QW)):
                        yield lambda q=q, c0=c0, c1=c1: nc.tensor.matmul(
                            y_ps[32 * q : 32 * q + C, c0:c1],
                            lw,
                            e_sb[:, jb * NI + q * QW + c0 : jb * NI + q * QW + c1],
                            start=False,
                            stop=last,
                            skip_group_check=True,
                            tile_position=(0, 32 * q),
                        )

            def emit(gen, k):
                for _ in range(k):
                    f = next(gen, None)
                    if f is None:
                        return
                    f()

            pb = iter(())
            for jb in range(NJB):
                # dot fills; A-fill MM order lets the early MMs run during
                # the previous jb's tail ACT chunks (see CHUNKS schedule).
                # passB MMs of jb-1 are spread into the PE FIFO at points
                # where ACT is busy, keeping the PE dense for HAM.
                fill(jb, "B", 0, 0, 512)                 # c0
                fill(jb, "A", 1024, 512 + 1024, 1024)    # c1 hi half
                fill(jb, "A", 0, 512, 1024)              # c1 lo half
                act(jb, 0)
                act(jb, 1)
                fill(jb, "B", 0, 2560, 512)              # c2
                emit(pb, 4)
                act(jb, 2)
                dve_sum(jb, 1)
                fill(jb, "A", 0, 3072, 2048)             # c3
                emit(pb, 4)
                act(jb, 3)
                fill(jb, "A", 0, 5120, 768)              # c4
                emit(pb, 12)
                act(jb, 4)
                dve_sum(jb, 3)
                nw_build(jb)
                pb = passb_mms(jb, last=(jb == NJB - 1))
            emit(pb, 12)

            nc.vector.tensor_copy(y_sb[:, :], y_ps[:, 0:QW])
            nc.sync.dma_start(yq[:, :], y_sb[:, :])
            nc.sync.dma_start(sout[:, :], s_red[:, :])

    nc.compile()
    return nc


def _split3(a):
    """3-way bf16 split: a ~ h + m + l to ~24 mantissa bits."""
    bf = ml_dtypes.bfloat16
    h = a.astype(bf)
    r1 = a - h.astype(np.float32)
    m = r1.astype(bf)
    l = (r1 - m.astype(np.float32)).astype(bf)
    return h, m, l


def kernel(I, U):
    global LAST_RESULT
    if "nc" not in _compiled:
        _compiled["nc"] = _build()
    nc = _compiled["nc"]

    I = np.asarray(I, np.float32)
    U = np.asarray(U, np.float32)

    g = np.arange(XD, dtype=np.float32)
    gx, gy, gz = np.meshgrid(g, g, g, indexing="ij")
    coords = np.stack([gx, gy, gz], 0).reshape(3, N)
    bf = ml_dtypes.bfloat16

    in_maps = []
    host = []
    for k in range(8):
        b, r = divmod(k, 4)
        feats = np.concatenate(
            [coords / ALPHA, I[b].reshape(3, N) / BETA], 0
        ).astype(np.float32)  # [6, N]
        sq = (feats.astype(np.float64) ** 2).sum(0)
        shalf = (-0.5 * sq).astype(np.float32)
        fh, fm, fl = _split3(feats)
        s1, s2, s3 = _split3(shalf)

        # rhs: all i (padded to NI); pad cols killed via NEG in the s1 row
        frhs = np.zeros((39, NI), bf)
        frhs[:, :N] = np.concatenate(
            [fh, fm, fh, fl, fh, fm, s1[None], s2[None], s3[None]], 0
        )
        frhs[36, N:] = bf(NEG)

        # lhs: own 1536 j rows; pad j killed via bias NEG
        gj = JW * r + np.arange(JW)
        valid = gj < N
        gjv = gj[valid]
        one = np.ones((1, len(gjv)), bf)
        flhs = np.zeros((39, JW), bf)
        flhs[:, valid] = np.concatenate(
            [
                fh[:, gjv], fh[:, gjv], fm[:, gjv], fh[:, gjv], fl[:, gjv],
                fm[:, gjv], one, one, one,
            ],
            0,
        )

        bpad = np.full(JW, NEG, np.float32)
        bpad[valid] = shalf[gjv]
        biasj = bpad.reshape(NJB, 128).T.copy()  # [128, NJB]

        uf = U[b].reshape(C, N).astype(np.float64)
        uf = uf - uf.max(0, keepdims=True)
        e = np.exp(uf)
        H1 = (e / e.sum(0, keepdims=True)).astype(np.float32)  # [C, N]
        hpad = np.zeros((C, JW), np.float32)
        hpad[:, valid] = H1[:, gjv]
        h1in = np.concatenate(
            [hpad[c].reshape(NJB, 128).T for c in range(C)], axis=1
        ).copy()  # [128, C*NJB]

        in_maps.append(
            {"flhs": flhs, "frhs": frhs, "biasj": biasj, "h1": h1in}
        )
        host.append((H1, valid, gjv))

    res = bass_utils.run_bass_kernel_spmd(
        nc, in_maps, core_ids=list(range(8)), trace=TRACE
    )
    LAST_RESULT = res

    loss = 0.0
    for b in range(B):
        Yb = np.zeros((C, NI), np.float64)
        s_full = np.zeros(N, np.float64)
        for r in range(4):
            k = 4 * b + r
            o = res.results[k]["yq"].astype(np.float64)  # [128, QW]
            for q in range(4):
                Yb[:, q * QW : (q + 1) * QW] += o[32 * q : 32 * q + C, :]
            sk = res.results[k]["sout"].astype(np.float64)  # [128, NJB]
            H1, valid, gjv = host[k]
            s_full[gjv] = sk.T.reshape(JW)[valid]
        H1 = host[4 * b][0].astype(np.float64)
        n = 1.0 / np.sqrt(s_full + EPS)
        acc = np.zeros(N, np.float64)
        for c in range(C):
            acc += Yb[c, :N] * (1.0 - H1[c])
        loss += float((n * acc).sum())
    return np.float32(loss)


# revision 11
# speedup vs baseline: 1.3119x; 1.0004x over previous
"""CRF loss (dense Gaussian bilateral filter) on 8 Trainium2 NeuronCores.

Math: feats f_i = (coords/ALPHA ++ I/BETA), K[i,j] = exp(-0.5||f_i-f_j||^2),
s = K @ 1, n = (s+EPS)^-1/2, H = softmax(U), v_c = n*H_c, Y_c = K v_c:
    loss = sum_c sum_i n_i Y_c,i (1 - H_c,i)
(uses sum_c H_c = 1 so the n^T K n term folds into the C channels).

Sharding (j-sharding, collective-free): cores 0-3 -> batch 0, 4-7 -> batch 1.
Each core owns 12 j-blocks of 128 (1536 rows of the padded-6144 contraction
dim) and streams ALL i (5888 padded). Row-sums s_j = sum_i K[j,i] are then
locally complete: they ride for free on the exp pass via the ScalarE
activation accum_out port. n = rsqrt(s) is computed on the otherwise-idle
VectorE (bit-trick seed + 2 Newton steps), so ScalarE does nothing but the
46*128-per-block exps, which are the kernel's critical path. Pass B
(Y_partial[c,i] = sum_{own j} w_j K[j,i], w = n*H_c) interleaves into the
PE's slack under the ACT-bound pass A, with the 4-plane output packed into
PSUM partition col-groups (i-quarters at partitions 32q..32q+3). The host
sums the 4 per-core partial Y's per batch and does the final f64 reduction.
"""

import numpy as np
import ml_dtypes

import concourse.bass as bass
import concourse.bacc as bacc
import concourse.tile as tile
import concourse.mybir as mybir
import concourse.bass_utils as bass_utils
from concourse.hw_specs import get_activation_tables

ALPHA = 5.0
BETA = 5.0
EPS = 1e-20

B = 2
C = 4
XD = YD = ZD = 18
N = XD * YD * ZD          # 5832
NI = 5888                 # i (free) padded to 46*128
JW = 1536                 # j rows per core (12 blocks of 128)
NJB = 12                  # own j blocks
NEG = -120.0              # pad bias => exp -> 0
QW = NI // 4              # 1472, i-quarter width for PSUM col-group packing
MAGIC = 0x5F3759DF        # fast inverse sqrt seed

F32 = mybir.dt.float32
I32 = mybir.dt.int32
BF16 = mybir.dt.bfloat16

# per-jb PSUM chunk schedule: (buf, ioff, width); A=2048 (4 banks), B=512
# (1 bank). Order chosen so every fill lands in the ACT-busy window of the
# preceding chunks (see pipeline notes in the session log).
CHUNKS = [
    ("B", 0, 512),
    ("A", 512, 2048),
    ("B", 2560, 512),
    ("A", 3072, 2048),
    ("A", 5120, 768),
]

TRACE = False
LAST_RESULT = None

_compiled = {}


def _build():
    nc = bacc.Bacc("TRN2", target_bir_lowering=False, debug=False, num_devices=8)

    flhs = nc.dram_tensor("flhs", [39, JW], BF16, kind="ExternalInput")
    frhs = nc.dram_tensor("frhs", [39, NI], BF16, kind="ExternalInput")
    biasj = nc.dram_tensor("biasj", [128, NJB], F32, kind="ExternalInput")
    h1 = nc.dram_tensor("h1", [128, C * NJB], F32, kind="ExternalInput")
    yq = nc.dram_tensor("yq", [128, QW], F32, kind="ExternalOutput")
    sout = nc.dram_tensor("sout", [128, NJB], F32, kind="ExternalOutput")

    with tile.TileContext(nc) as tc:
        with (
            tc.tile_pool(name="const", bufs=1) as cp,
            tc.tile_pool(name="epool", bufs=1) as ep,
            tc.tile_pool(name="ypsum", bufs=1, space="PSUM") as yp,
            tc.tile_pool(name="apsum", bufs=1, space="PSUM") as ap_,
            tc.tile_pool(name="bpsum", bufs=1, space="PSUM") as bp_,
        ):
            flhs_sb = cp.tile([39, JW], BF16)
            frhs_sb = cp.tile([39, NI], BF16)
            bias_sb = cp.tile([128, NJB], F32)
            h1_sb = cp.tile([128, C * NJB], F32)
            warm_sb = cp.tile([128, 512], BF16)
            s_acc = cp.tile([128, 5 * NJB], F32)
            s_red = cp.tile([128, NJB], F32)
            r_sb = cp.tile([128, NJB], F32)
            t1_sb = cp.tile([128, NJB], F32)
            w_sb = cp.tile([128, C * NJB], BF16)
            y_sb = cp.tile([128, QW], F32)
            dum_sb = cp.tile([128, 1], BF16)
            e_sb = ep.tile([128, NJB * NI], BF16)

            y_ps = yp.tile([128, 1536], F32)   # 3 banks; quarters in 0:1472
            a_ps = ap_.tile([128, 2048], F32)  # 4 banks
            b_ps = bp_.tile([128, 512], F32)   # 1 bank

            # ---- input DMA, chunked so jb0's first fills start early ----
            nc.sync.dma_start(flhs_sb[:], flhs[:])
            nc.sync.dma_start(frhs_sb[:, 0:2560], frhs[:, 0:2560])
            nc.sync.dma_start(bias_sb[:], biasj[:])
            nc.sync.dma_start(frhs_sb[:, 2560:NI], frhs[:, 2560:NI])
            nc.sync.dma_start(h1_sb[:], h1[:])
            nc.vector.memset(warm_sb[:], 0.0)
            nc.vector.memset(y_ps[:, :], 0.0)

            # Preload the exp table set so no ACT table switch mid-kernel.
            _tabs = list(get_activation_tables("gen3"))
            _nlx = _tabs.index("natural_log_exp_and_others")
            nc.scalar.add_instruction(
                mybir.InstLoadActFuncSet(
                    name=f"I-{nc.next_id()}", act_func_set_id=_nlx
                )
            )

            # ---- PE warmup: >=6us of sustained matmuls flip HAM to 2.4 GHz
            # (measured on this box: flip after ~14 back-to-back 512-col MMs)
            NWARM = 16
            for i in range(NWARM):
                nc.tensor.matmul(
                    b_ps[:, :],
                    warm_sb[:, 0:128],
                    warm_sb[:, 0:512],
                    start=(i == 0),
                    stop=(i == NWARM - 1),
                )

            h1_v = h1_sb[:, :].rearrange("p (c m) -> p m c", m=NJB)

            def fill(jb, buf, bufoff, ioff, width):
                """dot MMs for one 512-aligned span into PSUM buf."""
                lw = flhs_sb[:, 128 * jb : 128 * (jb + 1)]
                dst = a_ps if buf == "A" else b_ps
                done = 0
                while done < width:
                    w = min(512, width - done)
                    nc.tensor.matmul(
                        dst[:, bufoff + done : bufoff + done + w],
                        lw,
                        frhs_sb[:, ioff + done : ioff + done + w],
                        start=True,
                        stop=True,
                    )
                    done += w

            def act(jb, ci):
                # accum_out drops ACT to ~1.0 GHz and adds a ~341ns
                # READ_ACCUMULATOR -- only worth it on the small chunks;
                # the idle DVE sums the two 2048-col chunks instead.
                buf, ioff, width = CHUNKS[ci]
                src = a_ps if buf == "A" else b_ps
                acc = (
                    s_acc[:, 5 * jb + ci : 5 * jb + ci + 1]
                    if ci in (0, 2, 4)
                    else None
                )
                nc.scalar.activation(
                    e_sb[:, jb * NI + ioff : jb * NI + ioff + width],
                    src[:, 0:width],
                    mybir.ActivationFunctionType.Exp,
                    bias=bias_sb[:, jb : jb + 1],
                    scale=1.0,
                    accum_out=acc,
                )

            def dve_sum(jb, ci):
                _, ioff, width = CHUNKS[ci]
                e = e_sb[:, jb * NI + ioff : jb * NI + ioff + width]
                nc.vector.tensor_reduce(
                    s_acc[:, 5 * jb + ci : 5 * jb + ci + 1],
                    e,
                    op=mybir.AluOpType.add,
                    axis=mybir.AxisListType.X,
                )

            def nw_build(jb):
                """s -> n -> w on VectorE: fast-rsqrt + 2 Newton steps."""
                t = s_red[:, jb : jb + 1]
                nc.vector.tensor_reduce(
                    t,
                    s_acc[:, 5 * jb : 5 * jb + 5],
                    op=mybir.AluOpType.add,
                    axis=mybir.AxisListType.X,
                )
                # pad-j rows have s=0; rsqrt Newton would overflow -> NaN.
                # Valid s >= 247 and the host never reads pad rows.
                nc.vector.tensor_scalar_max(t, t, 1.0)
                r_i = r_sb[:, jb : jb + 1].bitcast(I32)
                nc.vector.tensor_scalar(
                    r_i,
                    t.bitcast(I32),
                    1,
                    -1,
                    op0=mybir.AluOpType.logical_shift_right,
                    op1=mybir.AluOpType.bitwise_xor,
                )
                nc.vector.tensor_scalar_add(r_i, r_i, MAGIC + 1)
                r = r_sb[:, jb : jb + 1]
                t1 = t1_sb[:, jb : jb + 1]
                for _ in range(2):
                    nc.vector.tensor_mul(t1, r, r)
                    nc.vector.tensor_mul(t1, t1, t)
                    nc.vector.tensor_scalar(
                        t1,
                        t1,
                        -0.5,
                        1.5,
                        op0=mybir.AluOpType.mult,
                        op1=mybir.AluOpType.add,
                    )
                    nc.vector.tensor_mul(r, r, t1)
                nc.vector.tensor_mul(
                    w_sb[:, C * jb : C * (jb + 1)],
                    h1_v[:, jb, :],
                    r.broadcast_to([128, C]),
                )

            def passb_mms(jb, last):
                """12 quarter MMs: Y[32q:32q+4, :] += w_jb^T E_jb."""
                lw = w_sb[:, C * jb : C * (jb + 1)]
                for q in range(4):
                    for c0, c1 in ((0, 512), (512, 1024), (1024, QW)):
                        yield lambda q=q, c0=c0, c1=c1: nc.tensor.matmul(
                            y_ps[32 * q : 32 * q + C, c0:c1],
                            lw,
                            e_sb[:, jb * NI + q * QW + c0 : jb * NI + q * QW + c1],
                            start=False,
                            stop=last,
                            skip_group_check=True,
                            tile_position=(0, 32 * q),
                        )

            def emit(gen, k):
                for _ in range(k):
                    f = next(gen, None)
                    if f is None:
                        return
                    f()

            # passB lags TWO jb behind the dots: its weights are then always
            # long-ready, so the MMs never stall the PE FIFO and instead
            # fill the PE idle windows (keeps HAM from re-throttling).
            pbq = []
            for jb in range(NJB):
                pb = pbq.pop(0) if len(pbq) > 1 else iter(())
                # dot fills; A-fill MM order lets the early MMs run during
                # the previous jb's tail ACT chunks (see CHUNKS schedule).
                fill(jb, "B", 0, 0, 512)                 # c0
                fill(jb, "A", 1024, 512 + 1024, 1024)    # c1 hi half
                fill(jb, "A", 0, 512, 1024)              # c1 lo half
                emit(pb, 4)
                act(jb, 0)
                act(jb, 1)
                fill(jb, "B", 0, 2560, 512)              # c2
                emit(pb, 4)
                act(jb, 2)
                dve_sum(jb, 1)
                fill(jb, "A", 0, 3072, 2048)             # c3
                emit(pb, 4)
                act(jb, 3)
                fill(jb, "A", 0, 5120, 768)              # c4
                emit(pb, 12)
                act(jb, 4)
                dve_sum(jb, 3)
                nw_build(jb)
                pbq.append(passb_mms(jb, last=(jb == NJB - 1)))
            for pb in pbq:
                emit(pb, 12)

            nc.vector.tensor_copy(y_sb[:, :], y_ps[:, 0:QW])
            nc.sync.dma_start(yq[:, :], y_sb[:, :])
            nc.sync.dma_start(sout[:, :], s_red[:, :])

    nc.compile()
    return nc


def _split3(a):
    """3-way bf16 split: a ~ h + m + l to ~24 mantissa bits."""
    bf = ml_dtypes.bfloat16
    h = a.astype(bf)
    r1 = a - h.astype(np.float32)
    m = r1.astype(bf)
    l = (r1 - m.astype(np.float32)).astype(bf)
    return h, m, l


def kernel(I, U):
    global LAST_RESULT
    if "nc" not in _compiled:
        _compiled["nc"] = _build()
    nc = _compiled["nc"]

    I = np.asarray(I, np.float32)
    U = np.asarray(U, np.float32)

    g = np.arange(XD, dtype=np.float32)
    gx, gy, gz = np.meshgrid(g, g, g, indexing="ij")
    coords = np.stack([gx, gy, gz], 0).reshape(3, N)
    bf = ml_dtypes.bfloat16

    in_maps = []
    host = []
    for k in range(8):
        b, r = divmod(k, 4)
        feats = np.concatenate(
            [coords / ALPHA, I[b].reshape(3, N) / BETA], 0
        ).astype(np.float32)  # [6, N]
        sq = (feats.astype(np.float64) ** 2).sum(0)
        shalf = (-0.5 * sq).astype(np.float32)
        fh, fm, fl = _split3(feats)
        s1, s2, s3 = _split3(shalf)

        # rhs: all i (padded to NI); pad cols killed via NEG in the s1 row
        frhs = np.zeros((39, NI), bf)
        frhs[:, :N] = np.concatenate(
            [fh, fm, fh, fl, fh, fm, s1[None], s2[None], s3[None]], 0
        )
        frhs[36, N:] = bf(NEG)

        # lhs: own 1536 j rows; pad j killed via bias NEG
        gj = JW * r + np.arange(JW)
        valid = gj < N
        gjv = gj[valid]
        one = np.ones((1, len(gjv)), bf)
        flhs = np.zeros((39, JW), bf)
        flhs[:, valid] = np.concatenate(
            [
                fh[:, gjv], fh[:, gjv], fm[:, gjv], fh[:, gjv], fl[:, gjv],
                fm[:, gjv], one, one, one,
            ],
            0,
        )

        bpad = np.full(JW, NEG, np.float32)
        bpad[valid] = shalf[gjv]
        biasj = bpad.reshape(NJB, 128).T.copy()  # [128, NJB]

        uf = U[b].reshape(C, N).astype(np.float64)
        uf = uf - uf.max(0, keepdims=True)
        e = np.exp(uf)
        H1 = (e / e.sum(0, keepdims=True)).astype(np.float32)  # [C, N]
        hpad = np.zeros((C, JW), np.float32)
        hpad[:, valid] = H1[:, gjv]
        h1in = np.concatenate(
            [hpad[c].reshape(NJB, 128).T for c in range(C)], axis=1
        ).copy()  # [128, C*NJB]

        in_maps.append(
            {"flhs": flhs, "frhs": frhs, "biasj": biasj, "h1": h1in}
        )
        host.append((H1, valid, gjv))

    res = bass_utils.run_bass_kernel_spmd(
        nc, in_maps, core_ids=list(range(8)), trace=TRACE
    )
    LAST_RESULT = res

    loss = 0.0
    for b in range(B):
        Yb = np.zeros((C, NI), np.float64)
        s_full = np.zeros(N, np.float64)
        for r in range(4):
            k = 4 * b + r
            o = res.results[k]["yq"].astype(np.float64)  # [128, QW]
            for q in range(4):
                Yb[:, q * QW : (q + 1) * QW] += o[32 * q : 32 * q + C, :]
            sk = res.results[k]["sout"].astype(np.float64)  # [128, NJB]
            H1, valid, gjv = host[k]
            s_full[gjv] = sk.T.reshape(JW)[valid]
        H1 = host[4 * b][0].astype(np.float64)
        n = 1.0 / np.sqrt(s_full + EPS)
        acc = np.zeros(N, np.float64)
        for c in range(C):
            acc += Yb[c, :N] * (1.0 - H1[c])
        loss += float((n * acc).sum())
    return np.float32(loss)
